# revision 17
# baseline (speedup 1.0000x reference)
"""Trainium2 Bass kernel for nn_MixModule (moe_routing).

Math: the reference collapses (linear in the one-hot `weights`) to
    y = x @ W_eff.T + b_eff,   W_eff = sum_o w_o W_o,  b_eff = sum_o w_o b_o.

Sharding: data-parallel, 16384 tokens per core across 8 NeuronCores; W/b
replicated; no cross-core communication.  fp16 wire dtype (gate is 2e-2,
measured rel err 3.6e-4): the host pre-casts and pre-transposes x to
[d, tokens] and upcasts/transposes y back — 4 MiB in + 4 MiB out per core.

Queue scheduling (measured on this part): SWDGE (gpsimd-issued) DMAs
preempt the HWDGE rings rather than round-robin.  So the continuous read
stream lives on the sync HWDGE ring (queued up front, full rate), and the
bursty drain-paced stores are issued via gpsimd/SWDGE — each store
preempts the read stream for ~0.6-2.4us as it lands, giving read+write
duplex (~400+ GB/s combined HBM) without a starvation phase.

Compute: PE runs one matmul per <=512-token group (lhsT = W_eff^T
stationary) into a ring of four 2-bank PSUM supers (depth-4 keeps
mm -> drain -> mm off the critical path).  ~5.4us of warm-up dummy matmuls
make the HAM warm flip deterministic (a short prefix leaves the PE at
1.2 GHz all run: measured +4.8us).  PSUM->SBUF drains (bias + fp16 cast)
go per 1024-token sub-chunk, alternating DVE (tensor_scalar) / ScalarE
(activation); fp32-PSUM caps each at 1x so the split keeps drain
throughput above chunk supply.  Chunk sizes taper at both ends to get the
first store out early and shorten the final load->mm->drain->store chain.
Flat (strided) DRAM layouts: chunk-contiguous measured slower.
"""

import numpy as np

import concourse.bass as bass
import concourse.mybir as mybir
from concourse.bass_utils import run_bass_kernel_spmd

B, S, D = 16, 8192, 128
N_CORES = 8
T = B * S // N_CORES          # tokens per core = 16384
CPAD = 256                    # consts columns prepended to x^T (fp16 elems)
SIZES = [512] + [2048] * 7 + [1024, 512]   # store chunk token counts
assert sum(SIZES) == T
OFFS = [sum(SIZES[:c]) for c in range(len(SIZES))]
N_CHUNKS = len(SIZES)
LOADS = [(sum(SIZES[:c]), SIZES[c]) for c in range(len(SIZES))]  # per-chunk
N_LOADS = len(LOADS)
GT = 512                      # tokens per matmul (one PSUM bank: 512 f32)
SUB = 1024                    # tokens per drain op (one PSUM super: 2 banks)
N_SUP = 4                     # PSUM supers (ring)
N_WARM = 50                   # HAM warm-up dummy matmuls (~107ns spacing)
F16 = mybir.dt.float16
F32 = mybir.dt.float32

# global sub-chunk list: (chunk, token_offset, size)
SUBS = []
for _c, _sz in enumerate(SIZES):
    _off = OFFS[_c]
    while _sz > 0:
        _s = min(SUB, _sz)
        SUBS.append((_c, _off, _s))
        _off += _s
        _sz -= _s
N_SUBS = len(SUBS)
SUB_FIRST = {c: min(i for i, s in enumerate(SUBS) if s[0] == c) for c in range(N_CHUNKS)}
SUB_LAST = {c: max(i for i, s in enumerate(SUBS) if s[0] == c) for c in range(N_CHUNKS)}
SUB_NEED_LOAD = {}
for _s, (_c, _toff, _sz) in enumerate(SUBS):
    for _l, (_lo, _lsz) in enumerate(LOADS):
        if _lo < _toff + _sz and _toff < _lo + _lsz:
            SUB_NEED_LOAD.setdefault(_l, _s)


def _build_bass():
    nc = bass.Bass(enable_partition_id=False)
    x = nc.dram_tensor("x", [128, CPAD + T], F16, kind="ExternalInput")
    y = nc.dram_tensor("y", [128, T], F16, kind="ExternalOutput")

    import contextlib
    with contextlib.ExitStack() as ctx:
        sem = lambda name: ctx.enter_context(nc.semaphore(name))
        s_id = sem("s_id")
        s_in = [sem(f"s_in{c}") for c in range(N_LOADS)]
        s_st = [sem(f"s_st{c}") for c in range(N_CHUNKS)]
        s_mm = sem("s_mm")      # counts completed sub-chunks of matmuls
        s_dv = sem("s_dv")      # even-sub drains (DVE)
        s_ac = sem("s_ac")      # odd-sub drains (ScalarE)

        xsb = ctx.enter_context(nc.sbuf_tensor("xsb", [128, CPAD + T], F16))
        ysb = ctx.enter_context(nc.sbuf_tensor("ysb", [128, T], F16))
        dum = ctx.enter_context(nc.sbuf_tensor("dum", [128, 128], F16))
        ps = [
            ctx.enter_context(nc.psum_tensor(f"ps{i}", [128, SUB], F32))
            for i in range(N_SUP)
        ]

        wT_ap = xsb[:, 0:128]                          # [d, f] fp16
        bias_ap = xsb[:, 128:130].bitcast(F32)         # [f, 1] f32

        def drain_wait(eng, s):
            # wait until sub-chunk s's drain has completed
            if s % 2 == 0:
                eng.wait_ge(s_dv, s // 2 + 1)
            else:
                eng.wait_ge(s_ac, (s + 1) // 2)

        with nc.Block(no_gpsimd_drain=True) as block:

            @block.sync
            def _(sp: bass.BassEngine):
                # the continuous read stream: all loads queued up front on
                # the HWDGE ring; chunk 0 carries the consts
                sp.dma_start(out=xsb[:, 0:CPAD + SIZES[0]],
                             in_=x[:, 0:CPAD + SIZES[0]]).then_inc(s_in[0], 16)
                for c in range(1, N_LOADS):
                    lo, hi = CPAD + OFFS[c], CPAD + OFFS[c] + SIZES[c]
                    sp.dma_start(out=xsb[:, lo:hi], in_=x[:, lo:hi]
                                 ).then_inc(s_in[c], 16)

            @block.gpsimd
            def _(gp: bass.BassGpSimd):
                gp.memset(dum[:, :], 0.0).then_inc(s_id)
                # stores via SWDGE: each preempts the read stream briefly,
                # so writes overlap reads instead of queuing behind them
                for c in range(N_CHUNKS):
                    drain_wait(gp, SUB_LAST[c])
                    if SUB_LAST[c] != SUB_FIRST[c]:
                        drain_wait(gp, SUB_FIRST[c])
                    gp.dma_start(out=y[:, OFFS[c]:OFFS[c] + SIZES[c]],
                                 in_=ysb[:, OFFS[c]:OFFS[c] + SIZES[c]]
                                 ).then_inc(s_st[c], 16)
                for c in range(N_CHUNKS):
                    gp.wait_ge(s_st[c], 16)

            @block.tensor
            def _(pe: bass.BassTensorEngine):
                pe.wait_ge(s_id, 1)
                for _ in range(N_WARM):
                    pe.matmul(out=ps[0][:, 0:128], lhsT=dum[:, :],
                              rhs=dum[:, :], start=True, stop=True)
                need_load = {v: k for k, v in SUB_NEED_LOAD.items()}
                for s, (c, toff, sz) in enumerate(SUBS):
                    if s in need_load:
                        pe.wait_ge(s_in[need_load[s]], 16)
                    if s >= N_SUP:
                        drain_wait(pe, s - N_SUP)   # ps[s % N_SUP] free
                    ngrp = sz // GT
                    for j in range(ngrp):
                        t0 = toff + j * GT
                        mm = pe.matmul(
                            out=ps[s % N_SUP][:, j * GT:(j + 1) * GT],
                            lhsT=wT_ap,
                            rhs=xsb[:, CPAD + t0:CPAD + t0 + GT],
                            start=True,
                            stop=True,
                        )
                        if j == ngrp - 1:
                            mm.then_inc(s_mm)

            @block.vector
            def _(dve: bass.BassVectorEngine):
                for s in range(0, N_SUBS, 2):
                    c, toff, sz = SUBS[s]
                    dve.wait_ge(s_mm, s + 1)
                    dve.tensor_scalar_add(
                        out=ysb[:, toff:toff + sz],
                        in0=ps[s % N_SUP][:, 0:sz],
                        scalar1=bias_ap,
                    ).then_inc(s_dv)

            @block.scalar
            def _(act: bass.BassScalarEngine):
                for s in range(1, N_SUBS, 2):
                    c, toff, sz = SUBS[s]
                    act.wait_ge(s_mm, s + 1)
                    act.activation(
                        out=ysb[:, toff:toff + sz],
                        in_=ps[s % N_SUP][:, 0:sz],
                        func=mybir.ActivationFunctionType.Identity,
                        bias=bias_ap,
                    ).then_inc(s_ac)

    return nc


_NC_CACHE = {}


def _get_nc():
    if "nc" not in _NC_CACHE:
        _NC_CACHE["nc"] = _build_bass()
    return _NC_CACHE["nc"]


def _make_in_maps(x, W, b, weights):
    x = np.asarray(x, dtype=np.float32)
    W = np.asarray(W, dtype=np.float32)
    b = np.asarray(b, dtype=np.float32)
    weights = np.asarray(weights, dtype=np.float32)

    w_eff = np.einsum("o,ofd->fd", weights.astype(np.float64), W.astype(np.float64))
    wT = w_eff.T.astype(np.float16)                               # [d, f]
    b_eff = (weights.astype(np.float64) @ b.astype(np.float64)).astype(np.float32)

    consts = np.zeros((128, CPAD), dtype=np.float16)
    consts[:, 0:128] = wT
    consts[:, 128:130] = b_eff.reshape(128, 1).view(np.float16)   # f32 bit pair

    xT = x.reshape(N_CORES, T, D).astype(np.float16).transpose(0, 2, 1)
    xfull = np.empty((N_CORES, 128, CPAD + T), dtype=np.float16)
    xfull[:, :, :CPAD] = consts
    xfull[:, :, CPAD:] = xT
    return [{"x": xfull[i]} for i in range(N_CORES)]


def _assemble(results):
    yT = np.stack([results[i]["y"] for i in range(N_CORES)])      # [8, 128, T]
    return yT.transpose(0, 2, 1).reshape(B, S, D).astype(np.float32)


def kernel(x, W, b, weights):
    nc = _get_nc()
    res = run_bass_kernel_spmd(nc, _make_in_maps(x, W, b, weights),
                               list(range(N_CORES)))
    return _assemble(res.results)


def kernel_profiled(x, W, b, weights, **kw):
    """Same as kernel() but traces; returns (y, BassKernelResults)."""
    nc = _get_nc()
    res = run_bass_kernel_spmd(nc, _make_in_maps(x, W, b, weights),
                               list(range(N_CORES)), trace=True, **kw)
    return _assemble(res.results), res


# revision 18
# speedup vs baseline: 1.0930x; 1.0930x over previous
"""Trainium2 Bass kernel for nn_MixModule (moe_routing).

Math: the reference computes outs[b,s,o,f] = sum_d x[b,s,d]*W[o,f,d] + b[o,f],
then y = sum_o weights[o]*outs[...,o,:].  This is linear in `weights`, so it
collapses to a single affine map:

    W_eff[f,d] = sum_o weights[o] * W[o,f,d]
    b_eff[f]   = sum_o weights[o] * b[o,f]
    y          = x @ W_eff.T + b_eff

Sharding: data-parallel over the batch axis, 2 batches (16384 tokens) per core
across 8 NeuronCores; W/b/weights replicated; no cross-core communication.

Wire compression (the problem is HBM-bound; the rel-err gate is 2e-2):
  - x is quantized host-side to int8 with one global scale folded into W:
    x_q = round(x * 127/xmax), W' = W_eff * xmax/127.  SWDGE (gpsimd-issued)
    DMAs cast int8 -> fp16 inline during the transfer, so HBM read traffic
    is 1 byte/elem (2.1 MiB/core) while SBUF receives ready-to-matmul fp16.
  - y goes back as fp16 (4.1 MiB/core); the host transposes/upcasts.
  - Measured end-to-end rel err 1.05e-2 (gate 2e-2; int8 quantization
    dominates, fp16 W/y rounding ~3e-4).
  - x is host-pre-transposed to [d, tokens], so there is no on-chip
    transpose at all.

Engine/queue layout:
  - gpsimd: 8 cast-load DMAs (SWDGE queue), fine-grained at both ends of
    the schedule so the first drain starts early and the final
    load->mm->drain->store serial chain is short.
  - sync: consts DMA (fp16 W' cols 0-127, f32 bias bitcast in cols 128-129)
    then the 10 stores, each waiting its chunk's drain sems (HWDGE ring).
    Loads and stores are on different queues, so the SDMA engines
    round-robin them: the write stream overlaps the (now small) read
    stream at the measured 400+ GB/s duplex rate.
  - PE: one matmul per <=512-token group (lhsT = W' stationary) into a ring
    of four 2-bank PSUM supers (depth-4 keeps mm -> drain -> mm off the
    critical path).  ~5.4us of warm-up dummy matmuls make the HAM warm
    flip deterministic (a short prefix leaves the PE at 1.2 GHz all run:
    measured +4.8us).
  - Drains (PSUM->SBUF + bias + fp16 cast) per 1024-token sub-chunk,
    alternating DVE (tensor_scalar, even subs) / ScalarE (activation with
    bias, odd subs); fp32-PSUM sources cap each at 1x, so the split is
    what keeps drain throughput above chunk supply.
  - Flat (strided) DRAM layouts throughout: chunk-contiguous measured
    slower (~280 vs ~347 GB/s; worse DRAM bank spread).
"""

import numpy as np

import concourse.bass as bass
import concourse.mybir as mybir
from concourse.bass_utils import run_bass_kernel_spmd

B, S, D = 16, 8192, 128
N_CORES = 8
T = B * S // N_CORES          # tokens per core = 16384
SIZES = [512] + [2048] * 7 + [1024, 512]   # store chunk token counts
assert sum(SIZES) == T
OFFS = [sum(SIZES[:c]) for c in range(len(SIZES))]
N_CHUNKS = len(SIZES)
# load grid (int8 cast-loads via SWDGE, ~1us issue each on gpsimd):
# fine at both ends, coarse in the middle
LOADS = [(0, 512), (512, 3072), (3584, 3072), (6656, 3072), (9728, 3072),
         (12800, 2048), (14848, 1024), (15872, 512)]
assert sum(sz for _, sz in LOADS) == T
N_LOADS = len(LOADS)
GT = 512                      # tokens per matmul (one PSUM bank: 512 f32)
SUB = 1024                    # tokens per drain op (one PSUM super: 2 banks)
N_SUP = 4                     # PSUM supers (ring)
N_WARM = 50                   # HAM warm-up dummy matmuls (~107ns spacing)
CW = 256                      # consts tensor columns (fp16)
F16 = mybir.dt.float16
F32 = mybir.dt.float32
I8 = mybir.dt.int8

# global sub-chunk list: (chunk, token_offset, size)
SUBS = []
for _c, _sz in enumerate(SIZES):
    _off = OFFS[_c]
    while _sz > 0:
        _s = min(SUB, _sz)
        SUBS.append((_c, _off, _s))
        _off += _s
        _sz -= _s
N_SUBS = len(SUBS)
SUB_FIRST = {c: min(i for i, s in enumerate(SUBS) if s[0] == c) for c in range(N_CHUNKS)}
SUB_LAST = {c: max(i for i, s in enumerate(SUBS) if s[0] == c) for c in range(N_CHUNKS)}
# first sub index that needs load l (load boundaries all lie on sub starts)
SUB_NEED_LOAD = {}
for _s, (_c, _toff, _sz) in enumerate(SUBS):
    for _l, (_lo, _lsz) in enumerate(LOADS):
        if _lo < _toff + _sz and _toff < _lo + _lsz:
            SUB_NEED_LOAD.setdefault(_l, _s)


def _build_bass():
    nc = bass.Bass(enable_partition_id=False)
    x = nc.dram_tensor("x", [128, T], I8, kind="ExternalInput")
    consts = nc.dram_tensor("consts", [128, CW], F16, kind="ExternalInput")
    y = nc.dram_tensor("y", [128, T], F16, kind="ExternalOutput")

    import contextlib
    with contextlib.ExitStack() as ctx:
        sem = lambda name: ctx.enter_context(nc.semaphore(name))
        s_id = sem("s_id")
        s_cn = sem("s_cn")
        s_in = [sem(f"s_in{c}") for c in range(N_LOADS)]
        s_st = [sem(f"s_st{c}") for c in range(N_CHUNKS)]
        s_mm = sem("s_mm")      # counts completed sub-chunks of matmuls
        s_dv = sem("s_dv")      # even-sub drains (DVE)
        s_ac = sem("s_ac")      # odd-sub drains (ScalarE)

        xsb = ctx.enter_context(nc.sbuf_tensor("xsb", [128, T], F16))
        ysb = ctx.enter_context(nc.sbuf_tensor("ysb", [128, T], F16))
        csb = ctx.enter_context(nc.sbuf_tensor("csb", [128, CW], F16))
        dum = ctx.enter_context(nc.sbuf_tensor("dum", [128, 128], F16))
        ps = [
            ctx.enter_context(nc.psum_tensor(f"ps{i}", [128, SUB], F32))
            for i in range(N_SUP)
        ]

        wT_ap = csb[:, 0:128]                          # [d, f] fp16 (scaled)
        bias_ap = csb[:, 128:130].bitcast(F32)         # [f, 1] f32

        def drain_wait(eng, s):
            # wait until sub-chunk s's drain has completed
            if s % 2 == 0:
                eng.wait_ge(s_dv, s // 2 + 1)
            else:
                eng.wait_ge(s_ac, (s + 1) // 2)

        with nc.Block(no_gpsimd_drain=True) as block:

            @block.gpsimd
            def _(gp: bass.BassGpSimd):
                gp.memset(dum[:, :], 0.0).then_inc(s_id)
                # int8 -> fp16 cast happens inside the SDMA datapath
                for l, (lo, lsz) in enumerate(LOADS):
                    gp.dma_start(out=xsb[:, lo:lo + lsz], in_=x[:, lo:lo + lsz]
                                 ).then_inc(s_in[l], 16)

            @block.sync
            def _(sp: bass.BassEngine):
                sp.dma_start(out=csb[:, :], in_=consts[:, :]).then_inc(s_cn, 16)
                for c in range(N_CHUNKS):
                    drain_wait(sp, SUB_LAST[c])
                    if SUB_LAST[c] != SUB_FIRST[c]:
                        drain_wait(sp, SUB_FIRST[c])
                    sp.dma_start(out=y[:, OFFS[c]:OFFS[c] + SIZES[c]],
                                 in_=ysb[:, OFFS[c]:OFFS[c] + SIZES[c]]
                                 ).then_inc(s_st[c], 16)
                for c in range(N_CHUNKS):
                    sp.wait_ge(s_st[c], 16)

            @block.tensor
            def _(pe: bass.BassTensorEngine):
                pe.wait_ge(s_id, 1)
                for _ in range(N_WARM):
                    pe.matmul(out=ps[0][:, 0:128], lhsT=dum[:, :],
                              rhs=dum[:, :], start=True, stop=True)
                pe.wait_ge(s_cn, 16)
                need_load = {v: k for k, v in SUB_NEED_LOAD.items()}
                for s, (c, toff, sz) in enumerate(SUBS):
                    if s in need_load:
                        pe.wait_ge(s_in[need_load[s]], 16)
                    if s >= N_SUP:
                        drain_wait(pe, s - N_SUP)   # ps[s % N_SUP] free
                    ngrp = sz // GT
                    for j in range(ngrp):
                        t0 = toff + j * GT
                        mm = pe.matmul(
                            out=ps[s % N_SUP][:, j * GT:(j + 1) * GT],
                            lhsT=wT_ap,
                            rhs=xsb[:, t0:t0 + GT],
                            start=True,
                            stop=True,
                        )
                        if j == ngrp - 1:
                            mm.then_inc(s_mm)

            @block.vector
            def _(dve: bass.BassVectorEngine):
                for s in range(0, N_SUBS, 2):
                    c, toff, sz = SUBS[s]
                    dve.wait_ge(s_mm, s + 1)
                    dve.tensor_scalar_add(
                        out=ysb[:, toff:toff + sz],
                        in0=ps[s % N_SUP][:, 0:sz],
                        scalar1=bias_ap,
                    ).then_inc(s_dv)

            @block.scalar
            def _(act: bass.BassScalarEngine):
                for s in range(1, N_SUBS, 2):
                    c, toff, sz = SUBS[s]
                    act.wait_ge(s_mm, s + 1)
                    act.activation(
                        out=ysb[:, toff:toff + sz],
                        in_=ps[s % N_SUP][:, 0:sz],
                        func=mybir.ActivationFunctionType.Identity,
                        bias=bias_ap,
                    ).then_inc(s_ac)

    return nc


_NC_CACHE = {}


def _get_nc():
    if "nc" not in _NC_CACHE:
        _NC_CACHE["nc"] = _build_bass()
    return _NC_CACHE["nc"]


def _make_in_maps(x, W, b, weights):
    x = np.asarray(x, dtype=np.float32)
    W = np.asarray(W, dtype=np.float32)
    b = np.asarray(b, dtype=np.float32)
    weights = np.asarray(weights, dtype=np.float32)

    w_eff = np.einsum("o,ofd->fd", weights.astype(np.float64), W.astype(np.float64))
    b_eff = (weights.astype(np.float64) @ b.astype(np.float64)).astype(np.float32)

    # global-scale int8 quantization of x; the scale folds into W
    xmax = float(np.abs(x).max())
    if xmax == 0.0:
        xmax = 1.0
    xq = np.clip(np.round(x * (127.0 / xmax)), -127, 127).astype(np.int8)
    wT = (w_eff.T * (xmax / 127.0)).astype(np.float16)            # [d, f]

    consts = np.zeros((128, CW), dtype=np.float16)
    consts[:, 0:128] = wT
    consts[:, 128:130] = b_eff.reshape(128, 1).view(np.float16)   # f32 bit pair

    xT = np.ascontiguousarray(
        xq.reshape(N_CORES, T, D).transpose(0, 2, 1))             # int8 [8,128,T]
    return [{"x": xT[i], "consts": consts} for i in range(N_CORES)]


def _assemble(results):
    yT = np.stack([results[i]["y"] for i in range(N_CORES)])      # [8, 128, T]
    return yT.transpose(0, 2, 1).reshape(B, S, D).astype(np.float32)


def kernel(x, W, b, weights):
    nc = _get_nc()
    res = run_bass_kernel_spmd(nc, _make_in_maps(x, W, b, weights),
                               list(range(N_CORES)))
    return _assemble(res.results)


def kernel_profiled(x, W, b, weights, **kw):
    """Same as kernel() but traces; returns (y, BassKernelResults)."""
    nc = _get_nc()
    res = run_bass_kernel_spmd(nc, _make_in_maps(x, W, b, weights),
                               list(range(N_CORES)), trace=True, **kw)
    return _assemble(res.results), res
